# revision 42
# baseline (speedup 1.0000x reference)
"""Trainium2 Bass kernel for nn_Expansion (e3nn-style tensor-product expansion).

Math reformulation (verified against the jax reference):
  h   = silu(node_emb @ lw1 + lb1)                         [B,64]
  hb  = silu(node_emb @ bw1 + bb1)                         [B,64]
  x0  = feat[:,:128] @ W0 / sqrt(128)                      [B,16]
  x1k = feat[:,128+k::3] @ W1 / 8          (k=0,1,2)       [B,16]

The per-sample path contractions  r = sum_w w_path[b,w,:] * x[b,w]  with
w_path = (h @ lw2 + lb2) sliced, are a batched bilinear form

  r[b,p] = sum_{c,w} h'[b,c] x[b,w] M[(c,w), p],   h' = [h, 1]

which becomes a plain matmul over the outer product  z[b,(c,w)] = h'[b,c]*x[b,w]
(K = 65*16 = 1040) against reshaped weight matrices M built from lw2/lb2 on the
host.  This avoids materializing w = h@lw2 ([B,36864], ~600 MB) entirely.

Sharding: pure data parallel, batch 4096 -> 8 cores x 512.  Weights replicated.

Modes:
  - "fp8x3" (default): main matmuls in fp8e4m3 DoubleRow (K=256/instr,
    0.5 cyc/row). 3-pass error compensation keeps rel err ~5e-3:
      r*S = z8 @ R8a + z8 @ R8b + dz8 @ R8a (+ bias bf16)
    with R8a = fp8(R*S), R8b = fp8(R*S - R8a), z8 = fp8(z), dz8 = fp8(z - z8),
    S = 2^9 (puts R values in e4m3's normal range); 1/S folded into the
    PSUM->SBUF copies.  Prep MLPs/replications run bf16.
  - "bf16": the previous all-bf16 single-pass pipeline (fallback).

Device layout per core (B_c = 512):
  - Inputs pre-transposed ([cols, B_c]) so contractions land on partitions;
    small weights are host-packed into 3 blobs (DMA issue is ~650ns each).
  - z built per batch-HALF as DR-paired fp8 tiles via a fixed engine chain:
    DVE muls of partition-replicated h'/x tiles (PE replication against
    constant Gsel/Tsel selectors) -> ACT casts to fp8 -> DVE/Pool residual
    subtracts. Half 0 unblocks b-tiles 0/1 early; half 1 builds while their
    banks run on PE.
  - Main matmuls accumulate out[b_tile=128, N<=512] in PSUM over 12 DR
    fp8 matmuls + 1 bf16 bias matmul per bank; PSUM banks drain via
    1/S-scaled copies alternating ACT/DVE into an assembled [128, 6400]
    SBUF tile (strided APs do the 1o interleave), DMA'd out top-half-first
    per tile (bottom-first + split top on the last tile to shrink the tail).
"""

import sys

import numpy as np

sys.path.insert(0, "/opt/trn_rl_repo")

import ml_dtypes  # noqa: E402

B_TOTAL = 4096
N_CORES = 8
BC = B_TOTAL // N_CORES  # 512 samples per core
P = 128
NB = BC // P  # 4 b-tiles per core
C3 = 1.0 / np.sqrt(3.0)
S_SCALE = 512.0  # fp8 weight pre-scale (power of 2; exact to undo)

# matmul dtype mode: "fp8x3" | "bf16"
MM_MODE = "fp8x3"

F8NP = ml_dtypes.float8_e4m3
BFNP = ml_dtypes.bfloat16

_CACHE = {}


# --------------------------------------------------------------------------
# fp8x3 path
# --------------------------------------------------------------------------

def _build_program_fp8(skip_lb2):
    import concourse.tile as tile
    from concourse import bacc, mybir

    F32 = mybir.dt.float32
    BF = mybir.dt.bfloat16
    F8 = mybir.dt.float8e4

    nc = bacc.Bacc("TRN2", target_bir_lowering=False, debug=False,
                   num_devices=N_CORES)

    t = {}
    t["featT"] = nc.dram_tensor("featT", [320, BC], BF, kind="ExternalInput").ap()
    t["node_embT"] = nc.dram_tensor("node_embT", [P, BC], BF, kind="ExternalInput").ap()
    # packed small weights: pk128 = lw1|bw1|W0  [128, 144] bf16
    t["pk128"] = nc.dram_tensor("pk128", [P, 144], BF, kind="ExternalInput").ap()
    # pk65 = Gsel|BB|W1|Tsel(pad)  [65, 1024+1280+16+128] bf16
    t["pk65"] = nc.dram_tensor("pk65", [65, 2448], BF, kind="ExternalInput").ap()
    # biases stay f32 (ACT bias APs): lb1|bb1  [64, 2]
    t["lbb"] = nc.dram_tensor("lbb", [64, 2], F32, kind="ExternalInput").ap()
    # DR-packed fp8 weight pairs: [p, Q, i, n] flattened to [128, 8*N]
    t["R0a"] = nc.dram_tensor("R0a", [P, 8 * 1280], F8, kind="ExternalInput").ap()
    t["R0b"] = nc.dram_tensor("R0b", [P, 8 * 1280], F8, kind="ExternalInput").ap()
    t["R1a"] = nc.dram_tensor("R1a", [P, 8 * 1024], F8, kind="ExternalInput").ap()
    t["R1b"] = nc.dram_tensor("R1b", [P, 8 * 1024], F8, kind="ExternalInput").ap()
    if not skip_lb2:
        t["R0t"] = nc.dram_tensor("R0t", [16, 1280], BF, kind="ExternalInput").ap()
        t["R1t"] = nc.dram_tensor("R1t", [16, 1024], BF, kind="ExternalInput").ap()
    t["out"] = nc.dram_tensor("out", [BC, 6400], F32, kind="ExternalOutput").ap()

    with tile.TileContext(nc) as tc:
        _emit_fp8(tc, t, skip_lb2, mybir, BF, F8, F32)

    nc.compile()
    return nc


def _emit_fp8(tc, t, skip_lb2, mybir, BF, F8, F32):
    nc = tc.nc
    from contextlib import ExitStack

    AF = mybir.ActivationFunctionType
    ALU = mybir.AluOpType
    DR = mybir.MatmulPerfMode.DoubleRow
    INV_S = float(1.0 / S_SCALE)

    with ExitStack() as ctx:
        wpool = ctx.enter_context(tc.tile_pool(name="weights", bufs=1))
        apool = ctx.enter_context(tc.tile_pool(name="acts", bufs=1))
        zpool = ctx.enter_context(tc.tile_pool(name="z", bufs=1))
        zbfp = ctx.enter_context(tc.tile_pool(name="zbf", bufs=3))
        opool = ctx.enter_context(tc.tile_pool(name="outs", bufs=3))
        pre_psum = ctx.enter_context(tc.tile_pool(name="pre_psum", bufs=1, space="PSUM"))
        prex_psum = ctx.enter_context(tc.tile_pool(name="prex_psum", bufs=3, space="PSUM"))
        main_psum = ctx.enter_context(tc.tile_pool(name="main_psum", bufs=4, space="PSUM"))

        # ---- SBUF tiles ----
        R0a_sb = wpool.tile([P, 4, 2, 1280], F8, tag="R0a")
        R0b_sb = wpool.tile([P, 4, 2, 1280], F8, tag="R0b")
        R1a_sb = wpool.tile([P, 4, 2, 1024], F8, tag="R1a")
        R1b_sb = wpool.tile([P, 4, 2, 1024], F8, tag="R1b")
        pk128_sb = wpool.tile([P, 144], BF, tag="pk128")
        pk65_sb = wpool.tile([65, 2448], BF, tag="pk65")
        lbb_sb = wpool.tile([64, 2], F32, tag="lbb")
        if not skip_lb2:
            R0t_sb = wpool.tile([16, 1280], BF, tag="R0t")
            R1t_sb = wpool.tile([16, 1024], BF, tag="R1t")

        # packed views
        lw1_sb = pk128_sb[:, 0:64]
        bw1_sb = pk128_sb[:, 64:128]
        W0_sb = pk128_sb[:, 128:144]
        G_sb = pk65_sb[:, 0:1024]
        BB_sb = pk65_sb[:, 1024:2304]
        W1_sb = pk65_sb[0:64, 2304:2320]
        T_sb = pk65_sb[0:16, 2320:2448]
        lb1_sb = lbb_sb[:, 0:1]
        bb1_sb = lbb_sb[:, 1:2]

        feats_sb = apool.tile([P, BC], BF, tag="feats")
        featv_sb = apool.tile([64, 3, BC], BF, tag="featv")
        emb_sb = apool.tile([P, BC], BF, tag="emb")

        # ---- input DMAs (batched; ~650ns issue each makes count matter) ----
        nc.sync.dma_start(emb_sb[:], t["node_embT"][:])
        nc.sync.dma_start(pk128_sb[:], t["pk128"][:])
        nc.sync.dma_start(lbb_sb[:], t["lbb"][:])
        nc.sync.dma_start(pk65_sb[:], t["pk65"][:])
        nc.sync.dma_start(feats_sb[:], t["featT"][0:128])
        nc.sync.dma_start(
            featv_sb[:], t["featT"][128:320].rearrange("(k p) b -> p k b", k=3))
        if not skip_lb2:
            nc.sync.dma_start(R0t_sb[:], t["R0t"][:])
            nc.sync.dma_start(R1t_sb[:], t["R1t"][:])

        # big fp8 weights, split by the column blocks the matmul banks
        # consume, in (a, b) pass order
        r0av = t["R0a"].rearrange("p (q two n) -> p q two n", q=4, two=2)
        r0bv = t["R0b"].rearrange("p (q two n) -> p q two n", q=4, two=2)
        r1av = t["R1a"].rearrange("p (q two n) -> p q two n", q=4, two=2)
        r1bv = t["R1b"].rearrange("p (q two n) -> p q two n", q=4, two=2)
        # column-chunk order follows bank consumption: p00a/b (R0 cols
        # 0:1024), p01k* (R1 0:512), p11 (R0 1024:1280), p10i* (R1 512:1024)
        for c0, c1 in ((0, 512), (512, 1024)):
            nc.sync.dma_start(R0a_sb[:, :, :, c0:c1], r0av[:, :, :, c0:c1])
            nc.sync.dma_start(R0b_sb[:, :, :, c0:c1], r0bv[:, :, :, c0:c1])
        nc.sync.dma_start(R1a_sb[:, :, :, 0:512], r1av[:, :, :, 0:512])
        nc.sync.dma_start(R1b_sb[:, :, :, 0:512], r1bv[:, :, :, 0:512])
        nc.sync.dma_start(R0a_sb[:, :, :, 1024:1280], r0av[:, :, :, 1024:1280])
        nc.sync.dma_start(R0b_sb[:, :, :, 1024:1280], r0bv[:, :, :, 1024:1280])
        nc.sync.dma_start(R1a_sb[:, :, :, 512:1024], r1av[:, :, :, 512:1024])
        nc.sync.dma_start(R1b_sb[:, :, :, 512:1024], r1bv[:, :, :, 512:1024])

        # ---- tiny MLP heads: h', hb', x0, x1k  (contraction on partitions) --
        # dummy silu on a scratch tile first: preloads the ACT function table
        # (~1.3us) during the input DMAs instead of on the critical chain
        warm_sb = apool.tile([1, 2], F32, tag="warm")
        nc.vector.memset(warm_sb[:], 0.0)
        nc.scalar.activation(warm_sb[:], warm_sb[:], AF.Silu)

        ph = pre_psum.tile([64, BC], F32, tag="pre")
        nc.tensor.matmul(ph[:], lhsT=lw1_sb, rhs=emb_sb[:], start=True, stop=True)
        hp_sb = apool.tile([65, BC], BF, tag="hp")
        nc.scalar.activation(hp_sb[0:64, :], ph[:], AF.Silu, bias=lb1_sb)
        nc.vector.memset(hp_sb[64:65, :], 1.0)

        # ---- partition-replicated tiles for the z outer product ----
        #   xbc[t][p, b] = x_t[p % 16, b]        (Tsel)
        #   hbc[q][p, b] = h'[8q + p//16, b]     (Gsel)
        # hbc matmuls come right after hp (the t=0 z muls consume hbc[q] in
        # order); psum->sbuf replication copies alternate DVE/ACT (GPSIMD
        # cannot read PSUM) so the replication phase drains ~2x faster.
        def psum_copy(dst, src, on_act):
            if on_act:
                nc.scalar.copy(dst, src)
            else:
                nc.vector.tensor_copy(out=dst, in_=src)

        # Replicated tiles are produced per batch-HALF (h=0: b-tiles 0-1,
        # h=1: b-tiles 2-3) so the first half of the z build — and with it
        # the first output banks/DMAs — unblocks as early as possible.
        HB = BC // 2
        xs_sb = []
        xbc = [[None] * 2 for _ in range(4)]
        hbc = [[None] * 2 for _ in range(8)]

        def emit_x(tdx):
            px = prex_psum.tile([16, BC], F32, tag="px")
            if tdx == 0:
                nc.tensor.matmul(px[:], lhsT=W0_sb, rhs=feats_sb[:],
                                 start=True, stop=True)
            else:
                nc.tensor.matmul(px[:], lhsT=W1_sb, rhs=featv_sb[:, tdx - 1, :],
                                 start=True, stop=True)
            xf = apool.tile([16, BC], BF, name=f"xf{tdx}", tag=f"xf{tdx}")
            nc.scalar.copy(xf[:], px[:])
            xs_sb.append(xf)
            px_bc = prex_psum.tile([P, BC], F32, name=f"pxbc{tdx}", tag="px")
            nc.tensor.matmul(px_bc[:], lhsT=T_sb, rhs=xf[:],
                             start=True, stop=True)
            for h in range(2):
                xb = apool.tile([P, HB], BF, name=f"xbc{tdx}_{h}",
                                tag=f"xbc{tdx}_{h}")
                psum_copy(xb[:], px_bc[:, HB * h:HB * (h + 1)],
                          on_act=((tdx + h) % 2 == 1))
                xbc[tdx][h] = xb

        # x0 chain first (feats arrive before hp is ready), so xbc[0] is done
        # by the time the hbc replications drain and the t=0 muls can flow
        emit_x(0)
        for q in range(8):
            ph_bc = prex_psum.tile([P, BC], F32, name=f"phbc{q}", tag="px")
            nc.tensor.matmul(ph_bc[:], lhsT=G_sb[:, P * q:P * (q + 1)],
                             rhs=hp_sb[:], start=True, stop=True)
            for h in range(2):
                hb = apool.tile([P, HB], BF, name=f"hbc{q}_{h}",
                                tag=f"hbc{q}_{h}")
                psum_copy(hb[:], ph_bc[:, HB * h:HB * (h + 1)],
                          on_act=((q + h) % 2 == 1))
                hbc[q][h] = hb
        for tdx in range(1, 4):
            emit_x(tdx)

        pb = pre_psum.tile([64, BC], F32, tag="pre")
        nc.tensor.matmul(pb[:], lhsT=bw1_sb, rhs=emb_sb[:], start=True, stop=True)
        hbp_sb = apool.tile([65, BC], BF, tag="hbp")
        nc.scalar.activation(hbp_sb[0:64, :], pb[:], AF.Silu, bias=bb1_sb)
        nc.vector.memset(hbp_sb[64:65, :], 1.0)

        # ---- z build: z8 (fp8) + dz8 (fp8 residual), DR-paired planes ----
        # Built per batch-half: z8h[t][h][:, Q, i, :] = fp8(hbc[2Q+i] * xbc[t])
        # over b-columns [HB*h, HB*(h+1)); dz8 = fp8(zbf - z8). Work spread:
        # muls on DVE, casts split ACT/Pool, subs split Pool/DVE, so no single
        # engine queue serializes the build against the bank copies.
        z8h = [[None] * 2 for _ in range(4)]
        dz8h = [[None] * 2 for _ in range(4)]
        for tdx in range(4):
            for h in range(2):
                z8h[tdx][h] = zpool.tile([P, 4, 2, HB], F8,
                                         name=f"z8_{tdx}_{h}",
                                         tag=f"z8_{tdx}_{h}")
                dz8h[tdx][h] = zpool.tile([P, 4, 2, HB], F8,
                                          name=f"dz8_{tdx}_{h}",
                                          tag=f"dz8_{tdx}_{h}")

        def emit_z_group(tdx, h):
            zb = zbfp.tile([P, 8, HB], BF, name=f"zb{tdx}_{h}", tag="zb")
            for q in range(8):
                nc.vector.tensor_mul(out=zb[:, q, :], in0=hbc[q][h][:],
                                     in1=xbc[tdx][h][:])
            z8t, dz8t = z8h[tdx][h], dz8h[tdx][h]
            # disjoint engine chain per group: mul (DVE) -> cast (ACT) ->
            # sub (Pool); each lane only ever waits on the previous stage
            for Q in range(4):
                nc.scalar.copy(z8t[:, Q, :, :], zb[:, 2 * Q:2 * Q + 2, :])
            for Q in range(4):
                eng = nc.vector if Q % 2 == 0 else nc.gpsimd
                eng.tensor_sub(
                    out=dz8t[:, Q, :, :], in0=zb[:, 2 * Q:2 * Q + 2, :],
                    in1=z8t[:, Q, :, :])

        # ---- main matmuls + output assembly ----
        def accum(tdx, Ra, Rb, Rt, col0, ncols, bias_cols, j, psum_ap):
            h, boff = j // 2, (j % 2) * P
            bhl = slice(boff, boff + P)
            bsl = slice(P * j, P * (j + 1))
            nmm = 12 + (0 if skip_lb2 else 1) + (1 if bias_cols is not None else 0)
            idx = 0
            for zt, rt in ((z8h[tdx][h], Ra), (z8h[tdx][h], Rb),
                           (dz8h[tdx][h], Ra)):
                for Q in range(4):
                    idx += 1
                    nc.tensor.matmul(psum_ap,
                                     lhsT=zt[:, Q, :, bhl],
                                     rhs=rt[:, Q, :, col0:col0 + ncols],
                                     perf_mode=DR,
                                     start=(idx == 1), stop=(idx == nmm))
            if not skip_lb2:
                idx += 1
                nc.tensor.matmul(psum_ap,
                                 lhsT=xs_sb[tdx][:, bsl],
                                 rhs=Rt[:, col0:col0 + ncols],
                                 start=False, stop=(idx == nmm))
            if bias_cols is not None:
                idx += 1
                nc.tensor.matmul(psum_ap,
                                 lhsT=hbp_sb[:, bsl],
                                 rhs=BB_sb[:, bias_cols[0]:bias_cols[1]],
                                 start=False, stop=(idx == nmm))

        R0t_ = None if skip_lb2 else R0t_sb
        R1t_ = None if skip_lb2 else R1t_sb

        def tile_ctx(j):
            out_t = opool.tile([P, 6400], F32, name="out_t", tag="out_t")
            o3 = out_t.rearrange("p (r c) -> p r c", c=80)          # [128,80,80]
            top = o3[:, 0:32, :]                                     # [128,32,80]
            bot = out_t[:, 2560:6400].rearrange(
                "p (u i c) -> p u i c", i=3, c=80)                   # [128,16,3,80]
            return out_t, o3, top, bot

        cctr = [0]

        def scaled_copy(dst, src):
            # alternate ACT/DVE so neither queue backs up behind the z build
            cctr[0] += 1
            if cctr[0] % 2 == 0:
                nc.scalar.mul(dst, src, INV_S)
            else:
                nc.vector.tensor_scalar_mul(out=dst, in0=src, scalar1=INV_S)

        def bank_p00(half, o3, j):
            c0 = 512 * half
            p00 = main_psum.tile([P, 512], F32, name=f"p00{half}", tag="mp")
            accum(0, R0a_sb, R0b_sb, R0t_, c0, 512, (c0, c0 + 512), j, p00[:])
            scaled_copy(o3[:, 16 * half:16 * half + 16, 0:32],
                        p00[:].rearrange("p (u v) -> p u v", v=32))

        def bank_p01(k, top, j, split=False):
            # r01k -> blk01: out[u, 32+3v+k], u<32, v<16
            p01 = main_psum.tile([P, 512], F32, name=f"p01_{k}", tag="mp")
            accum(1 + k, R1a_sb, R1b_sb, R1t_, 0, 512, None, j, p01[:])
            dst = top[:, :, 32:80].rearrange(
                "p u (v jj) -> p u v jj", jj=3)[:, :, :, k]          # [128,32,16]
            src = p01[:].rearrange("p (u v) -> p u v", v=16)
            if split:
                # row-split so the final top DMA can go out in two pieces
                scaled_copy(dst[:, 0:16, :], src[:, 0:16, :])
                scaled_copy(dst[:, 16:32, :], src[:, 16:32, :])
            else:
                scaled_copy(dst, src)

        def bank_p11(bot, j):
            # r11 -> blk11 diagonal-in-(i,j): out[32+3u+i, 32+3v+i]
            p11 = main_psum.tile([P, 512], F32, name="p11", tag="mp")
            accum(0, R0a_sb, R0b_sb, R0t_, 1024, 256, (1024, 1280), j,
                  p11[:, 0:256])
            src11 = p11[:, 0:256].rearrange("p (u v) -> p u v", v=16)
            for i in range(3):
                dst = bot[:, :, i, 32:80].rearrange(
                    "p u (v jj) -> p u v jj", jj=3)[:, :, :, i]      # [128,16,16]
                scaled_copy(dst, src11)

        def bank_p10(i, bot, j):
            # r10i -> blk10: out[32+3u+i, v], u<16, v<32
            p10 = main_psum.tile([P, 512], F32, name=f"p10_{i}", tag="mp")
            accum(1 + i, R1a_sb, R1b_sb, R1t_, 512, 512, None, j, p10[:])
            dst = bot[:, :, i, 0:32]                                 # [128,16,32]
            src = p10[:].rearrange("p (u v) -> p u v", v=32)
            scaled_copy(dst, src)

        def emit_tile(j, ctx3=None):
            bsl = slice(P * j, P * (j + 1))
            out_t, o3, top, bot = ctx3 if ctx3 is not None else tile_ctx(j)
            if ctx3 is None:
                nc.gpsimd.memset(o3[:, 32:80, 32:80], 0.0)
            if j < NB - 1:
                bank_p00(0, o3, j)
                bank_p00(1, o3, j)
                for k in range(3):
                    bank_p01(k, top, j)
                nc.sync.dma_start(t["out"][bsl, 0:2560], out_t[:, 0:2560])
                bank_p11(bot, j)
                for i in range(3):
                    bank_p10(i, bot, j)
                nc.sync.dma_start(t["out"][bsl, 2560:6400], out_t[:, 2560:6400])
            else:
                # last tile runs bottom-half first so the final (tail) DMA is
                # the smaller top half
                bank_p11(bot, j)
                for i in range(3):
                    bank_p10(i, bot, j)
                nc.sync.dma_start(t["out"][bsl, 2560:6400], out_t[:, 2560:6400])
                bank_p00(0, o3, j)
                bank_p00(1, o3, j)
                for k in range(3):
                    bank_p01(k, top, j, split=True)
                nc.sync.dma_start(t["out"][bsl, 0:1280], out_t[:, 0:1280])
                nc.sync.dma_start(t["out"][bsl, 1280:2560], out_t[:, 1280:2560])

        # half-0 z groups unblock b-tiles 0/1; half-1 builds while those
        # tiles' banks run on PE, so the PE never waits on z again.
        # blk11 zero memsets are slotted into the Pool queue just-in-time:
        # after the subs they must not delay, before the bottom DMA needs them
        pre_ctx = [tile_ctx(j) for j in range(3)]
        for tdx in range(4):
            emit_z_group(tdx, 0)
        nc.gpsimd.memset(pre_ctx[0][1][:, 32:80, 32:80], 0.0)
        emit_tile(0, pre_ctx[0])
        emit_z_group(0, 1)
        emit_z_group(1, 1)
        nc.gpsimd.memset(pre_ctx[1][1][:, 32:80, 32:80], 0.0)
        emit_z_group(2, 1)
        emit_z_group(3, 1)
        nc.gpsimd.memset(pre_ctx[2][1][:, 32:80, 32:80], 0.0)
        emit_tile(1, pre_ctx[1])
        emit_tile(2, pre_ctx[2])
        emit_tile(3)


def _prepare_fp8(inputs):
    f32 = np.float32
    feat = np.asarray(inputs["feat"], dtype=f32)
    node_emb = np.asarray(inputs["node_emb"], dtype=f32)
    W0 = np.asarray(inputs["W0"], f32)
    W1 = np.asarray(inputs["W1"], f32)
    lw1 = np.asarray(inputs["lw1"], f32)
    lb1 = np.asarray(inputs["lb1"], f32)
    lw2 = np.asarray(inputs["lw2"], f32)
    lb2 = np.asarray(inputs["lb2"], f32)
    bw1 = np.asarray(inputs["bw1"], f32)
    bb1 = np.asarray(inputs["bb1"], f32)
    bw2 = np.asarray(inputs["bw2"], f32)
    bb2 = np.asarray(inputs["bb2"], f32)

    s16 = np.float32(1.0 / 16.0)
    sC = np.float32(C3 / 16.0)
    S = np.float32(S_SCALE)

    lw2p = np.concatenate([lw2, lb2[None]], axis=0)           # [65, 36864]
    M00 = lw2p[:, :16384].reshape(1040, 1024) * s16
    M11 = lw2p[:, 16384:20480].reshape(1040, 256) * sC
    M01 = lw2p[:, 20480:28672].reshape(1040, 512) * sC
    M10 = lw2p[:, 28672:36864].reshape(1040, 512) * sC
    R0 = np.concatenate([M00, M11], axis=1) * S               # [1040, 1280]
    R1 = np.concatenate([M01, M10], axis=1) * S               # [1040, 1024]
    BBf = np.concatenate([bw2, bb2[None]], axis=0)            # [65, 1280]
    BB = (np.concatenate([BBf[:, :1024] * s16, BBf[:, 1024:] * sC], axis=1)
          * S).astype(BFNP)

    def pack_dr(Rm):
        # rows (q p) -> [p, Q, i, n] flattened; q = 2Q + i
        N = Rm.shape[1]
        return np.ascontiguousarray(
            Rm.reshape(4, 2, P, N).transpose(2, 0, 1, 3).reshape(P, 8 * N))

    R0a8 = R0[0:1024].astype(F8NP)
    R0b8 = (R0[0:1024] - R0a8.astype(f32)).astype(F8NP)
    R1a8 = R1[0:1024].astype(F8NP)
    R1b8 = (R1[0:1024] - R1a8.astype(f32)).astype(F8NP)
    R0a = pack_dr(R0a8)
    R0b = pack_dr(R0b8)
    R1a = pack_dr(R1a8)
    R1b = pack_dr(R1b8)

    skip_lb2 = not bool(np.any(lb2))
    R0t = np.ascontiguousarray(R0[1024:1040]).astype(BFNP)
    R1t = np.ascontiguousarray(R1[1024:1040]).astype(BFNP)

    W0s = (W0 * np.float32(1.0 / np.sqrt(128.0))).astype(BFNP)
    W1s = (W1 * np.float32(1.0 / 8.0)).astype(BFNP)
    lw1b = lw1.astype(BFNP)
    bw1b = bw1.astype(BFNP)
    lbb = np.ascontiguousarray(
        np.stack([lb1, bb1], axis=1).astype(f32))              # [64, 2]

    Gsel = np.zeros((65, 1024), np.float32)
    for q in range(8):
        for c8 in range(8):
            Gsel[8 * q + c8, 128 * q + 16 * c8:128 * q + 16 * (c8 + 1)] = 1.0
    Tsel = np.zeros((16, 128), np.float32)
    for w in range(16):
        Tsel[w, w::16] = 1.0

    # packed small-weight blobs (single DMA each)
    pk128 = np.ascontiguousarray(
        np.concatenate([lw1b, bw1b, W0s], axis=1))             # [128, 144]
    pk65 = np.zeros((65, 2448), BFNP)
    pk65[:, 0:1024] = Gsel.astype(BFNP)
    pk65[:, 1024:2304] = BB
    pk65[0:64, 2304:2320] = W1s
    pk65[0:16, 2320:2448] = Tsel.astype(BFNP)
    pk65 = np.ascontiguousarray(pk65)

    in_maps = []
    for i in range(N_CORES):
        sl = slice(i * BC, (i + 1) * BC)
        fs = feat[sl]
        featT = np.ascontiguousarray(
            np.concatenate(
                [fs[:, :128], fs[:, 128::3], fs[:, 129::3], fs[:, 130::3]],
                axis=1).T).astype(BFNP)                        # [320, BC]
        embT = np.ascontiguousarray(node_emb[sl].T).astype(BFNP)  # [128, BC]
        m = {
            "featT": featT,
            "node_embT": embT,
            "pk128": pk128, "pk65": pk65, "lbb": lbb,
            "R0a": R0a, "R0b": R0b, "R1a": R1a, "R1b": R1b,
        }
        if not skip_lb2:
            m["R0t"] = R0t
            m["R1t"] = R1t
        in_maps.append(m)
    return in_maps, skip_lb2


# --------------------------------------------------------------------------
# bf16 fallback path (previous pipeline)
# --------------------------------------------------------------------------

def _build_program_bf16(skip_lb2):
    import concourse.tile as tile
    from concourse import bacc, mybir

    F32 = mybir.dt.float32
    MM = mybir.dt.bfloat16
    AF = mybir.ActivationFunctionType

    nc = bacc.Bacc("TRN2", target_bir_lowering=False, debug=False,
                   num_devices=N_CORES)

    t = {}
    t["featT"] = nc.dram_tensor("featT", [320, BC], F32, kind="ExternalInput").ap()
    t["node_embT"] = nc.dram_tensor("node_embT", [P, BC], F32, kind="ExternalInput").ap()
    t["W0"] = nc.dram_tensor("W0", [P, 16], F32, kind="ExternalInput").ap()
    t["W1"] = nc.dram_tensor("W1", [64, 16], F32, kind="ExternalInput").ap()
    t["lw1"] = nc.dram_tensor("lw1", [P, 64], F32, kind="ExternalInput").ap()
    t["bw1"] = nc.dram_tensor("bw1", [P, 64], F32, kind="ExternalInput").ap()
    t["lb1c"] = nc.dram_tensor("lb1c", [64, 1], F32, kind="ExternalInput").ap()
    t["bb1c"] = nc.dram_tensor("bb1c", [64, 1], F32, kind="ExternalInput").ap()
    t["R0"] = nc.dram_tensor("R0", [1040, 1280], MM, kind="ExternalInput").ap()
    t["R1"] = nc.dram_tensor("R1", [1040, 1024], MM, kind="ExternalInput").ap()
    t["BB"] = nc.dram_tensor("BB", [65, 1280], MM, kind="ExternalInput").ap()
    t["Gsel"] = nc.dram_tensor("Gsel", [65, 1024], MM, kind="ExternalInput").ap()
    t["Tsel"] = nc.dram_tensor("Tsel", [16, 128], MM, kind="ExternalInput").ap()
    t["out"] = nc.dram_tensor("out", [BC, 6400], F32, kind="ExternalOutput").ap()

    with tile.TileContext(nc) as tc:
        _emit_bf16(tc, t, skip_lb2, mybir, MM, F32, AF)

    nc.compile()
    return nc


def _emit_bf16(tc, t, skip_lb2, mybir, MM, F32, AF):
    nc = tc.nc
    from contextlib import ExitStack

    with ExitStack() as ctx:
        wpool = ctx.enter_context(tc.tile_pool(name="weights", bufs=1))
        apool = ctx.enter_context(tc.tile_pool(name="acts", bufs=1))
        zpool = ctx.enter_context(tc.tile_pool(name="z", bufs=1))
        opool = ctx.enter_context(tc.tile_pool(name="outs", bufs=3))
        pre_psum = ctx.enter_context(tc.tile_pool(name="pre_psum", bufs=1, space="PSUM"))
        prex_psum = ctx.enter_context(tc.tile_pool(name="prex_psum", bufs=3, space="PSUM"))
        main_psum = ctx.enter_context(tc.tile_pool(name="main_psum", bufs=5, space="PSUM"))

        R0_sb = wpool.tile([P, 9, 1280], MM, tag="R0")
        R1_sb = wpool.tile([P, 9, 1024], MM, tag="R1")
        BB_sb = wpool.tile([65, 1280], MM, tag="BB")
        W0_sb = wpool.tile([P, 16], F32, tag="W0")
        W1_sb = wpool.tile([64, 16], F32, tag="W1")
        lw1_sb = wpool.tile([P, 64], F32, tag="lw1")
        bw1_sb = wpool.tile([P, 64], F32, tag="bw1")
        lb1_sb = wpool.tile([64, 1], F32, tag="lb1")
        bb1_sb = wpool.tile([64, 1], F32, tag="bb1")
        G_sb = wpool.tile([65, 1024], MM, tag="Gsel")
        T_sb = wpool.tile([16, 128], MM, tag="Tsel")

        feats_sb = apool.tile([P, BC], F32, tag="feats")
        featv_sb = [apool.tile([64, BC], F32, name=f"featv{k}", tag=f"featv{k}")
                    for k in range(3)]
        emb_sb = apool.tile([P, BC], F32, tag="emb")
        nc.sync.dma_start(emb_sb[:], t["node_embT"][:])
        nc.sync.dma_start(feats_sb[:], t["featT"][0:128])
        for k in range(3):
            nc.sync.dma_start(featv_sb[k][:], t["featT"][128 + 64 * k:192 + 64 * k])
        nc.sync.dma_start(lw1_sb[:], t["lw1"][:])
        nc.sync.dma_start(bw1_sb[:], t["bw1"][:])
        nc.sync.dma_start(W0_sb[:], t["W0"][:])
        nc.sync.dma_start(W1_sb[:], t["W1"][:])
        nc.sync.dma_start(lb1_sb[:], t["lb1c"][:])
        nc.sync.dma_start(bb1_sb[:], t["bb1c"][:])
        nc.sync.dma_start(G_sb[:], t["Gsel"][:])
        nc.sync.dma_start(T_sb[:], t["Tsel"][:])
        nc.sync.dma_start(BB_sb[:], t["BB"][:])

        r0v = t["R0"][0:1024].rearrange("(q p) n -> p q n", p=P)
        r1v = t["R1"][0:1024].rearrange("(q p) n -> p q n", p=P)
        for c0, c1 in ((0, 512), (512, 1024), (1024, 1280)):
            nc.sync.dma_start(R0_sb[:, 0:8, c0:c1], r0v[:, :, c0:c1])
        for c0, c1 in ((0, 512), (512, 1024)):
            nc.sync.dma_start(R1_sb[:, 0:8, c0:c1], r1v[:, :, c0:c1])
        if not skip_lb2:
            nc.sync.dma_start(R0_sb[0:16, 8, :], t["R0"][1024:1040])
            nc.sync.dma_start(R1_sb[0:16, 8, :], t["R1"][1024:1040])

        ph = pre_psum.tile([64, BC], F32, tag="pre")
        nc.tensor.matmul(ph[:], lhsT=lw1_sb[:], rhs=emb_sb[:], start=True, stop=True)
        hp_sb = apool.tile([65, BC], MM, tag="hp")
        nc.scalar.activation(hp_sb[0:64, :], ph[:], AF.Silu, bias=lb1_sb[:])
        nc.any.memset(hp_sb[64:65, :], 1.0)

        pb = pre_psum.tile([64, BC], F32, tag="pre")
        nc.tensor.matmul(pb[:], lhsT=bw1_sb[:], rhs=emb_sb[:], start=True, stop=True)
        hbp_sb = apool.tile([65, BC], MM, tag="hbp")
        nc.scalar.activation(hbp_sb[0:64, :], pb[:], AF.Silu, bias=bb1_sb[:])
        nc.any.memset(hbp_sb[64:65, :], 1.0)

        xs_sb = []
        for tdx in range(4):
            px = prex_psum.tile([16, BC], F32, tag="px")
            if tdx == 0:
                nc.tensor.matmul(px[:], lhsT=W0_sb[:], rhs=feats_sb[:],
                                 start=True, stop=True)
            else:
                nc.tensor.matmul(px[:], lhsT=W1_sb[:], rhs=featv_sb[tdx - 1][:],
                                 start=True, stop=True)
            xf = apool.tile([16, BC], MM, name=f"xf{tdx}", tag=f"xf{tdx}")
            nc.scalar.copy(xf[:], px[:])
            xs_sb.append(xf)
        xs_mm = xs_sb

        xbc = []
        for tdx in range(4):
            px_bc = prex_psum.tile([P, BC], F32, name=f"pxbc{tdx}", tag="px")
            nc.tensor.matmul(px_bc[:], lhsT=T_sb[:], rhs=xs_sb[tdx][:],
                             start=True, stop=True)
            xb = apool.tile([P, BC], MM, name=f"xbc{tdx}", tag=f"xbc{tdx}")
            nc.scalar.copy(xb[:], px_bc[:])
            xbc.append(xb)
        hbc = []
        for q in range(8):
            ph_bc = prex_psum.tile([P, BC], F32, name=f"phbc{q}", tag="px")
            nc.tensor.matmul(ph_bc[:], lhsT=G_sb[:, P * q:P * (q + 1)],
                             rhs=hp_sb[:], start=True, stop=True)
            hb = apool.tile([P, BC], MM, name=f"hbc{q}", tag=f"hbc{q}")
            nc.scalar.copy(hb[:], ph_bc[:])
            hbc.append(hb)
        z = [[None] * 8 for _ in range(4)]
        for tdx in range(4):
            for q in range(8):
                zt = zpool.tile([P, BC], MM, name=f"z{tdx}_{q}", tag=f"z{tdx}_{q}")
                nc.vector.tensor_mul(out=zt[:], in0=hbc[q][:], in1=xbc[tdx][:])
                z[tdx][q] = zt

        def accum2(tdx, rhs_sb, col0, ncols, bias_cols, bsl, psum_ap):
            nmm = 8 + (0 if skip_lb2 else 1) + (1 if bias_cols is not None else 0)
            idx = 0
            for q in range(8):
                idx += 1
                nc.tensor.matmul(psum_ap,
                                 lhsT=z[tdx][q][:, bsl],
                                 rhs=rhs_sb[:, q, col0:col0 + ncols],
                                 start=(idx == 1), stop=(idx == nmm))
            if not skip_lb2:
                idx += 1
                nc.tensor.matmul(psum_ap,
                                 lhsT=xs_mm[tdx][:, bsl],
                                 rhs=rhs_sb[0:16, 8, col0:col0 + ncols],
                                 start=False, stop=(idx == nmm))
            if bias_cols is not None:
                idx += 1
                nc.tensor.matmul(psum_ap,
                                 lhsT=hbp_sb[:, bsl],
                                 rhs=BB_sb[:, bias_cols[0]:bias_cols[1]],
                                 start=False, stop=(idx == nmm))

        for j in range(NB):
            bsl = slice(P * j, P * (j + 1))
            out_t = opool.tile([P, 6400], F32, name="out_t", tag="out_t")
            o3 = out_t.rearrange("p (r c) -> p r c", c=80)
            top = o3[:, 0:32, :]
            bot = out_t[:, 2560:6400].rearrange(
                "p (u i c) -> p u i c", i=3, c=80)

            nc.gpsimd.memset(o3[:, 32:80, 32:80], 0.0)

            p00a = main_psum.tile([P, 512], F32, name="p00a", tag="mp")
            accum2(0, R0_sb, 0, 512, (0, 512), bsl, p00a[:])
            nc.scalar.copy(o3[:, 0:16, 0:32],
                           p00a[:].rearrange("p (u v) -> p u v", v=32))
            p00b = main_psum.tile([P, 512], F32, name="p00b", tag="mp")
            accum2(0, R0_sb, 512, 512, (512, 1024), bsl, p00b[:])
            nc.scalar.copy(o3[:, 16:32, 0:32],
                           p00b[:].rearrange("p (u v) -> p u v", v=32))

            for k in range(3):
                p01 = main_psum.tile([P, 512], F32, name=f"p01_{k}", tag="mp")
                accum2(1 + k, R1_sb, 0, 512, None, bsl, p01[:])
                dst = top[:, :, 32:80].rearrange(
                    "p u (v jj) -> p u v jj", jj=3)[:, :, :, k]
                src = p01[:].rearrange("p (u v) -> p u v", v=16)
                if k == 0:
                    nc.scalar.copy(dst, src)
                else:
                    nc.vector.tensor_copy(out=dst, in_=src)

            nc.sync.dma_start(t["out"][bsl, 0:2560], out_t[:, 0:2560])

            p11 = main_psum.tile([P, 512], F32, name="p11", tag="mp")
            accum2(0, R0_sb, 1024, 256, (1024, 1280), bsl, p11[:, 0:256])
            src11 = p11[:, 0:256].rearrange("p (u v) -> p u v", v=16)
            for i in range(3):
                dst = bot[:, :, i, 32:80].rearrange(
                    "p u (v jj) -> p u v jj", jj=3)[:, :, :, i]
                nc.vector.tensor_copy(out=dst, in_=src11)

            for i in range(3):
                p10 = main_psum.tile([P, 512], F32, name=f"p10_{i}", tag="mp")
                accum2(1 + i, R1_sb, 512, 512, None, bsl, p10[:])
                dst = bot[:, :, i, 0:32]
                src = p10[:].rearrange("p (u v) -> p u v", v=32)
                if i == 0:
                    nc.scalar.copy(dst, src)
                else:
                    nc.vector.tensor_copy(out=dst, in_=src)

            nc.sync.dma_start(t["out"][bsl, 2560:6400], out_t[:, 2560:6400])


def _prepare_bf16(inputs):
    f32 = np.float32
    feat = np.ascontiguousarray(np.asarray(inputs["feat"], dtype=f32))
    node_emb = np.ascontiguousarray(np.asarray(inputs["node_emb"], dtype=f32))
    W0 = np.asarray(inputs["W0"], f32)
    W1 = np.asarray(inputs["W1"], f32)
    lw1 = np.asarray(inputs["lw1"], f32)
    lb1 = np.asarray(inputs["lb1"], f32)
    lw2 = np.asarray(inputs["lw2"], f32)
    lb2 = np.asarray(inputs["lb2"], f32)
    bw1 = np.asarray(inputs["bw1"], f32)
    bb1 = np.asarray(inputs["bb1"], f32)
    bw2 = np.asarray(inputs["bw2"], f32)
    bb2 = np.asarray(inputs["bb2"], f32)

    mmnp = BFNP
    s16 = np.float32(1.0 / 16.0)
    sC = np.float32(C3 / 16.0)

    lw2p = np.concatenate([lw2, lb2[None]], axis=0)
    M00 = lw2p[:, :16384].reshape(1040, 1024) * s16
    M11 = lw2p[:, 16384:20480].reshape(1040, 256) * sC
    M01 = lw2p[:, 20480:28672].reshape(1040, 512) * sC
    M10 = lw2p[:, 28672:36864].reshape(1040, 512) * sC
    R0 = np.ascontiguousarray(np.concatenate([M00, M11], axis=1)).astype(mmnp)
    R1 = np.ascontiguousarray(np.concatenate([M01, M10], axis=1)).astype(mmnp)
    BBf = np.concatenate([bw2, bb2[None]], axis=0)
    BB = np.ascontiguousarray(
        np.concatenate([BBf[:, :1024] * s16, BBf[:, 1024:] * sC], axis=1)
    ).astype(mmnp)

    W0s = np.ascontiguousarray(W0 * np.float32(1.0 / np.sqrt(128.0)))
    W1s = np.ascontiguousarray(W1 * np.float32(1.0 / 8.0))
    lb1c = np.ascontiguousarray(lb1[:, None])
    bb1c = np.ascontiguousarray(bb1[:, None])

    Gsel = np.zeros((65, 1024), np.float32)
    for q in range(8):
        for c8 in range(8):
            Gsel[8 * q + c8, 128 * q + 16 * c8:128 * q + 16 * (c8 + 1)] = 1.0
    Tsel = np.zeros((16, 128), np.float32)
    for w in range(16):
        Tsel[w, w::16] = 1.0
    Gsel = Gsel.astype(mmnp)
    Tsel = Tsel.astype(mmnp)

    skip_lb2 = not bool(np.any(lb2))

    in_maps = []
    for i in range(N_CORES):
        sl = slice(i * BC, (i + 1) * BC)
        fs = feat[sl]
        featT = np.ascontiguousarray(
            np.concatenate(
                [fs[:, :128], fs[:, 128::3], fs[:, 129::3], fs[:, 130::3]],
                axis=1).T)
        embT = np.ascontiguousarray(node_emb[sl].T)
        in_maps.append({
            "featT": featT,
            "node_embT": embT,
            "W0": W0s, "W1": W1s,
            "lw1": lw1, "bw1": bw1,
            "lb1c": lb1c, "bb1c": bb1c,
            "R0": R0, "R1": R1, "BB": BB, "Gsel": Gsel, "Tsel": Tsel,
        })
    return in_maps, skip_lb2


# --------------------------------------------------------------------------

def run(inputs, mode=None, trace=False):
    """Build (cached), run on 8 cores, gather. Returns (out, results)."""
    mode = mode or MM_MODE
    if mode == "fp8x3":
        in_maps, skip_lb2 = _prepare_fp8(inputs)
        key = (mode, skip_lb2)
        if key not in _CACHE:
            _CACHE[key] = _build_program_fp8(skip_lb2)
    else:
        in_maps, skip_lb2 = _prepare_bf16(inputs)
        key = (mode, skip_lb2)
        if key not in _CACHE:
            _CACHE[key] = _build_program_bf16(skip_lb2)
    nc = _CACHE[key]

    from concourse.bass_utils import run_bass_kernel_spmd
    res = run_bass_kernel_spmd(nc, in_maps, list(range(N_CORES)), trace=trace)
    out = np.concatenate(
        [res.results[i]["out"].reshape(BC, 80, 80) for i in range(N_CORES)],
        axis=0)
    return out.astype(np.float32), res


def kernel(**inputs):
    out, _ = run(inputs)
    return out
